# revision 37
# baseline (speedup 1.0000x reference)
"""Trainium2 Bass kernel for gated multi-head pair attention (AlphaFold-style).

Reference computation (B=1, N=256, C=128, H=4, DH=32):
    q = (q_data @ wq.T) * DH**-0.5        # [B,N,Nq,C]
    k = m_data @ wk.T ; v = m_data @ wv.T
    logits = einsum("bnqhd,bnkhd->bnhqk", q, k) + bias + nonbatched_bias
    weight = softmax(logits, axis=-1)
    wa = einsum("bnhqk,bnkhd->bnqhd", weight, v)
    g  = sigmoid(q_data @ wg.T + bg)
    out = (wa * g).reshape(...) @ wo.T + bo

Sharding: pure data-parallel across the 8 NeuronCores along the first
residue axis (N): core c owns rows [32c, 32c+32). Params + nonbatched_bias
replicated.

On-device structure (2-row macro-iterations, software-pipelined):
  - XT/MT = q_data[n].T / m_data[n].T in SBUF [C=128 part, 256 tok] (host
    pre-transposes; pure layout prep); DMA split per row across queues and
    prefetched two pairs ahead
  - projections batched per row pair (N=512 = one full PSUM bank each):
    q2/k2 into one 2-bank tile, g2 into another; e1 = exp(-(gpre+bg))
    batched for both rows
  - logitsT[h] = k_h @ q_h.T  k-major [128 ktok, 256 q]; 4 heads via PE
    row-tiling (K=32 at row groups 32h), two per 2-bank lg tile
  - exp fused on ScalarE: exp(KEY_SCALE*logits + bias_col[k]), then one
    wide DVE multiply by precomputed exp(nonbatched_bias) per (row, kc)
  - waU[h] = v_h.T @ expT and S[h] = sum_k expT via col-tiled (M=32)
    matmuls into each row's ws tile; emul+wa/s deferred one step to avoid
    PE-FIFO head-of-line blocking of the next exp's kq matmuls
  - gate+normalize via the exp/ln ACT table set only (one table load):
    wag = waU * sigmoid(gpre+bg)/S = waU * exp(-ln(S*(1+exp(-gpre-bg))))
  - out[qc] = wag[:,qc].T @ woT (+ bo) back to q-major; contiguous DMA out
  - PSUM (8 banks): "lg" rotation 2 bufs x 2 banks (proj + logit tiles) +
    per-row "ws" tiles 2 bufs x 2 banks (v -> wa/s -> out-proj reuse);
    fronts/epilogues interleave mid-pair so ScalarE gaps stay filled

Environment workarounds (this walrus build): one sem wait max per
instruction (_legalize_multiwaits splits extras onto same-engine NOPs);
two matmuls must never write different column ranges of the same PSUM bank
(device fault) so every matmul output gets a bank-exclusive region;
custom-DVE / GPSIMD tensor ops fail codegen, hence exp(-ln(x)) reciprocal.
"""

import os
import sys

sys.path.insert(0, "/opt/trn_rl_repo")

from contextlib import ExitStack

import numpy as np

import concourse.bass as bass
import concourse.tile as tile
from concourse import mybir
from concourse.bass_utils import run_bass_kernel_spmd

B, N, C, H = 1, 256, 128, 4
DH = C // H
KEY_SCALE = DH**-0.5
NCORES = 8
RPC = int(os.getenv("KRPC", str(N // NCORES)))  # rows per core
KLEVEL = int(os.getenv("KLEVEL", "99"))  # debug feature level
WITH_BO = True  # set by kernel() per-input; bo==0 skips the bias matmuls

F32 = mybir.dt.float32
BF16 = mybir.dt.bfloat16
FP16 = mybir.dt.float16

# dtype of the post-softmax path (exp weights, v, attention matmul)
EXP_DT = BF16

_CACHE = {}


def _legalize_multiwaits(nc, max_waits=1):
    """The walrus build here encodes at most one sem wait per instruction
    ("Too many sync wait commands" otherwise). Split excess waits onto
    freshly inserted Drain instructions on the same engine just before the
    multi-wait instruction (engines execute in order, so this is
    equivalent)."""
    n_fix = 0
    for f in nc.m.functions:
        for blk in f.blocks:
            changed = False
            new_insts = []
            for inst in blk.instructions:
                si = inst.sync_info
                ow = list(si.on_wait) if (si is not None and si.on_wait) else []
                if len(ow) > max_waits:
                    head, tail = ow[:-max_waits], ow[-max_waits:]
                    while head:
                        chunk, head = head[:max_waits], head[max_waits:]
                        d = mybir.InstNoOp(
                            name=f"I-mw{nc.next_id()}", ins=[], outs=[]
                        )
                        d.engine = inst.engine
                        d.sync_info = mybir.SyncInfo(
                            on_wait=list(chunk), on_update=[]
                        )
                        new_insts.append(d)
                        n_fix += 1
                    inst.sync_info = mybir.SyncInfo(
                        on_wait=list(tail),
                        on_update=list(si.on_update) if si.on_update else [],
                    )
                    changed = True
                new_insts.append(inst)
            if changed:
                blk.instructions = new_insts
    return n_fix


def _emit(ctx: ExitStack, tc: "tile.TileContext", t):
    nc = tc.nc

    const = ctx.enter_context(tc.tile_pool(name="const", bufs=1))

    def load_const(name, shape, dtype=F32):
        sb = const.tile(shape, dtype, name=name + "_sb")
        nc.sync.dma_start(sb, t[name].ap())
        return sb

    wq_sb = load_const("wqT", [C, C], BF16)
    wk_sb = load_const("wkT", [C, C], BF16)
    wv_sb = load_const("wvT", [C, C], BF16)
    wg_sb = load_const("wgT", [C, C], BF16)
    wo_sb = load_const("woT", [C, C], BF16)
    bo_sb = load_const("bo_row", [1, C], BF16)
    bgn_sb = load_const("bgn_col", [C, 1])
    bias_sb = load_const("bias_r", [128, 2 * RPC])
    nbt_sb = load_const("nbT", [128, 2 * H * N])

    ones1_sb = const.tile([1, C], BF16)
    nc.vector.memset(ones1_sb, 1.0)
    ones32_sb = const.tile([128, DH], EXP_DT)
    nc.vector.memset(ones32_sb, 1.0)

    # one-time: exp(nonbatched_bias), k-major layout [p, kc*1024 + h*256 + q]
    enb_sb = const.tile([128, 2 * H * N], EXP_DT)
    nc.scalar.activation(enb_sb, nbt_sb, mybir.ActivationFunctionType.Exp)

    io = ctx.enter_context(tc.tile_pool(name="io", bufs=4))
    sb = ctx.enter_context(tc.tile_pool(name="sb", bufs=6))
    exps = ctx.enter_context(tc.tile_pool(name="exps", bufs=10))
    # NOTE: this walrus/HW stack faults when two matmuls write different
    # column ranges of the same PSUM bank. Every matmul output below gets a
    # bank-aligned exclusive 512-col region (2-bank [128,1024] tiles hold two
    # 256-col results at cols 0 and 512); different-partition col-tiling and
    # same-region accumulation are safe.
    # Bank layout (8 banks total): "lg" rotation 2 bufs x 2 banks = 4 banks
    # (proj qk2/gv tiles + attention logit tiles flow through it, all freed
    # by early reads), and per-row "ws" tiles 2 bufs x 2 banks = 4 banks
    # (v -> wa/s -> out-projection sequentially reuse the same banks).
    lg_ps = ctx.enter_context(tc.tile_pool(name="lg_ps", bufs=2, space="PSUM"))
    sm_ps = ctx.enter_context(tc.tile_pool(name="sm_ps", bufs=2, space="PSUM"))

    xt_ap = t["xt"].ap()
    mt_ap = t["mt"].ap()
    out_ap = t["out"]

    MM = nc.tensor.matmul
    Act = mybir.ActivationFunctionType
    NB = 512  # psum bank stride (fp32 elems)

    def pair_view(tile_ap, width):
        # [128, 1024] tile -> [128, 2, width] view of cols [0:width] and
        # [512:512+width] (the two bank-aligned result slots)
        return tile_ap.rearrange("p (b x) -> p b x", b=2)[:, :, 0:width]

    def emit_dma(fr):
        r0 = fr["r0"]
        xt_sb = io.tile([128, 2 * N], BF16, tag="xt", name="xt_sb")
        mt_sb = io.tile([128, 2 * N], BF16, tag="mt", name="mt_sb")
        # one dma_start per row so the transfer spreads over more queues
        # (each [128,256] tile is ~128 descriptors on one queue)
        for rr in range(2):
            nc.sync.dma_start(xt_sb[:, N * rr : N * rr + N], xt_ap[r0 + rr])
            nc.sync.dma_start(mt_sb[:, N * rr : N * rr + N], mt_ap[r0 + rr])
        fr["xt"], fr["mt"] = xt_sb, mt_sb

    def emit_front_a(fr):
        """q/k projections + casts: the critical chain to the next pair's
        first kq matmuls.  Emitted as early as PSUM slots allow."""
        xt_sb, mt_sb = fr["xt"], fr["mt"]
        # projections for both rows in one matmul each (N=512 = 1 full
        # bank).  qk2 2-bank tile: q2 in bank0, k2 in bank1.
        qk2_ps = lg_ps.tile([128, 2 * NB], F32, tag="lg", name="qk2_ps")
        MM(qk2_ps[:, 0:NB], lhsT=wq_sb, rhs=xt_sb, start=True, stop=True)
        MM(qk2_ps[:, NB : 2 * NB], lhsT=wk_sb, rhs=mt_sb, start=True, stop=True)
        # qk_sb layout: [q_r0 | k_r0 | q_r1 | k_r1] each N wide.  One cast
        # per row (strided across the two banks) so row0's kq matmuls can
        # start after the first cast instead of after both.
        qk_sb = sb.tile([128, 4 * N], BF16, tag="qk", name="qk_sb")
        qk2v = qk2_ps.rearrange("p (b x) -> p b x", b=2)
        for rr in range(2):
            nc.vector.tensor_copy(
                qk_sb[:, 2 * N * rr : 2 * N * rr + 2 * N].rearrange(
                    "p (b x) -> p b x", b=2
                ),
                qk2v[:, :, N * rr : N * rr + N],
            )
        fr["qk"] = qk_sb

    def emit_front_b(fr):
        """gate projection + e1, and the v projections into the ws banks."""
        xt_sb, mt_sb = fr["xt"], fr["mt"]
        gv_ps = lg_ps.tile([128, 2 * NB], F32, tag="lg", name="gv_ps")
        MM(gv_ps[:, 0:NB], lhsT=wg_sb, rhs=xt_sb, start=True, stop=True)
        # sigmoid(gpre + bg) = 1/(1 + exp(-(gpre+bg))); only exp/ln are used
        # anywhere so the ACT table set loads exactly once.  Both rows in
        # one activation pass.
        e1_sb = sb.tile([128, 2 * N], F32, tag="e1", name="e1_sb")
        nc.scalar.activation(
            e1_sb, gv_ps[:, 0 : 2 * N], Act.Exp, bias=bgn_sb, scale=-1.0
        )
        fr["e1"] = e1_sb

        # v for both rows: [v_r0 | v_r1] each [128 tok, 2*C].  The v matmuls
        # write into each row's ws tile (bank0 cols 0:C, bank1 cols 0:C);
        # the same banks are later sequentially reused for wa/s and the out
        # projection (reuse is ordered by the intervening reads).
        v_sb = sb.tile([128, 4 * C], EXP_DT, tag="v", name="v_sb")
        ws_l = []
        for rr in range(2):
            ws_ps = sm_ps.tile([128, 2 * NB], F32, tag="ws", name=f"ws{rr}_ps")
            ws_l.append(ws_ps)
            MM(ws_ps[:, 0:C], lhsT=mt_sb[:, N * rr : N * rr + 128],
               rhs=wv_sb, start=True, stop=True)
            MM(ws_ps[:, NB : NB + C],
               lhsT=mt_sb[:, N * rr + 128 : N * rr + 256],
               rhs=wv_sb, start=True, stop=True)
            nc.vector.tensor_copy(
                v_sb.rearrange("p (r c x) -> p r c x", r=2, c=2)[:, rr, :, :],
                ws_ps.rearrange("p (b x) -> p b x", b=2)[:, :, 0:C],
            )
        fr["v"] = v_sb
        fr["ws"] = ws_l

    def emit_attention_step(fr, rr, r, kc):
        """Emit kq matmuls + exp for one (row, kc); return a deferred
        emitter for the emul + wa/s matmuls.  Deferring those past the next
        step's kq matmuls avoids PE-FIFO head-of-line blocking (wa/s wait
        on the DVE emul; kq behind them would stall the next exp)."""
        qk_sb, v_sb = fr["qk"], fr["v"]
        wa_ps = fr["ws"][rr][:, 0:N]
        s_ps = fr["ws"][rr][:, NB : NB + N]
        # e2 holds exp-weights for all 4 heads of this (row, kc):
        # [p, pr*512 + hh*256 + q] = [p, 256*h + q]
        e2_sb = exps.tile([128, 4 * N], EXP_DT, tag="e", name="e2_sb")
        for pr in range(2):  # head pair (2*pr, 2*pr+1)
            lg = lg_ps.tile(
                [128, 2 * NB], F32, tag="lg", name=f"lg{rr}{kc}{pr}"
            )
            for hh in range(2):
                h = 2 * pr + hh
                # logitsT[ktok, q] = k_h @ q_h.T ; K=DH=32 at row
                # group 32h -> heads run concurrently on the PE
                MM(
                    lg[:, NB * hh : NB * hh + N],
                    lhsT=qk_sb[
                        32 * h : 32 * h + 32,
                        (2 * rr + 1) * N + 128 * kc :
                        (2 * rr + 1) * N + 128 * kc + 128,
                    ],
                    rhs=qk_sb[
                        32 * h : 32 * h + 32, 2 * rr * N : (2 * rr + 1) * N
                    ],
                    start=True,
                    stop=True,
                    tile_position=(32 * h, 0),
                )
            nc.scalar.activation(
                e2_sb[:, 2 * N * pr : 2 * N * pr + 2 * N].rearrange(
                    "p (b x) -> p b x", b=2
                ),
                pair_view(lg, N),
                Act.Exp,
                bias=bias_sb[:, kc * RPC + r : kc * RPC + r + 1],
                scale=KEY_SCALE,
            )

        def deferred():
            # one wide multiply by exp(nonbatched_bias) for all 4 heads
            nc.vector.tensor_mul(
                e2_sb, e2_sb, enb_sb[:, 1024 * kc : 1024 * kc + 1024]
            )
            for h in range(H):
                # waU (unnormalized attention @ V), col-tiled by head
                MM(
                    wa_ps[32 * h : 32 * h + 32, :],
                    lhsT=v_sb[
                        :,
                        2 * C * rr + 128 * kc + 32 * h :
                        2 * C * rr + 128 * kc + 32 * h + 32,
                    ],
                    rhs=e2_sb[:, N * h : N * h + N],
                    start=(kc == 0),
                    stop=(kc == 1),
                    tile_position=(0, 32 * h),
                    skip_group_check=True,
                )
                # S = sum_k exp, broadcast to head's partition block
                MM(
                    s_ps[32 * h : 32 * h + 32, :],
                    lhsT=ones32_sb,
                    rhs=e2_sb[:, N * h : N * h + N],
                    start=(kc == 0),
                    stop=(kc == 1),
                    tile_position=(0, 32 * h),
                    skip_group_check=True,
                )

        return deferred

    def emit_epilogue_row(fr, rr, r):
        # wag = waU * sigmoid(gpre+bg) / S = waU * exp(-ln(S * (1 + e1)))
        e1_sb = fr["e1"]
        if True:
            ws_ps = fr["ws"][rr]
            d_sb = sb.tile([128, N], F32, tag="d", name=f"d{rr}_sb")
            nc.vector.scalar_tensor_tensor(
                d_sb,
                e1_sb[:, N * rr : N * rr + N],
                1.0,
                ws_ps[:, NB : NB + N],
                mybir.AluOpType.add,
                mybir.AluOpType.mult,
            )
            nc.scalar.activation(d_sb, d_sb, Act.Ln)
            rs_sb = sb.tile([128, N], F32, tag="rs", name=f"rs{rr}_sb")
            nc.scalar.activation(rs_sb, d_sb, Act.Exp, scale=-1.0)

            wag_sb = sb.tile([128, N], BF16, tag="wag", name=f"wag{rr}")
            nc.vector.tensor_mul(wag_sb, ws_ps[:, 0:N], rs_sb)

            out_sb = sb.tile([128, 2 * C], F32, tag="out", name=f"out{rr}")
            for qc in range(2):
                # out-projection reuses the row's ws banks (wa/s fully read
                # by wag/stt at this point)
                o_ps = ws_ps[:, NB * qc : NB * qc + C]
                MM(
                    o_ps,
                    lhsT=wag_sb[:, 128 * qc : 128 * qc + 128],
                    rhs=wo_sb,
                    start=True,
                    stop=not WITH_BO,
                    skip_group_check=True,
                )
                if WITH_BO:
                    MM(
                        o_ps,
                        lhsT=ones1_sb,
                        rhs=bo_sb,
                        start=False,
                        stop=True,
                        skip_group_check=True,
                    )
            # out copy on ScalarE: frees the DVE queue so emul/wag drain
            # sooner (PE stalls on those), and ACT has headroom vs PE
            nc.scalar.copy(
                out_sb.rearrange("p (b x) -> p b x", b=2),
                ws_ps.rearrange("p (b x) -> p b x", b=2)[:, :, 0:C],
            )
            # out dram [RPC, N, C]; tile is [p, qc, o] with q = qc*128 + p.
            # Last pair: split per qc chunk across two queues to halve the
            # end-of-kernel DMA drain tail (~256 descriptors per row).
            if r >= RPC - 2:
                for qc in range(2):
                    dst = bass.AP(
                        out_ap, r * N * C + qc * 128 * C, [[C, 128], [1, C]]
                    )
                    nc.sync.dma_start(dst, out_sb[:, C * qc : C * qc + C])
            else:
                dst = bass.AP(
                    out_ap, r * N * C, [[C, 128], [128 * C, 2], [1, C]]
                )
                nc.sync.dma_start(dst, out_sb)

    # Fully linearized software pipeline over steps (row, kc).  Per pair:
    #   kqA expA | wasD_prev | kqB expB | wasA | front_a' | kqC expC |
    #   wasB | front_b' | kqD expD | wasC | epi(r0) | ...next pair...
    # front_a' (q/k proj + casts) starts as soon as PSUM slots free
    # mid-pair; epilogues interleave as ACT fills; DMA is prefetched two
    # pairs ahead and split per row across queues.
    NPAIR = RPC // 2
    fronts = [{"r0": 2 * p} for p in range(NPAIR)]
    emit_dma(fronts[0])
    if NPAIR > 1:
        emit_dma(fronts[1])
    emit_front_a(fronts[0])
    emit_front_b(fronts[0])
    pending = None
    epi_pending = None
    for p in range(NPAIR):
        cur = fronts[p]
        for step in range(4):
            rr, kc = divmod(step, 2)
            nxt = emit_attention_step(cur, rr, cur["r0"] + rr, kc)
            if pending is not None:
                pending()
            pending = nxt
            if epi_pending is not None:
                emit_epilogue_row(*epi_pending)
                epi_pending = None
            if step == 1 and p + 1 < NPAIR:
                if p + 2 < NPAIR:
                    emit_dma(fronts[p + 2])
                emit_front_a(fronts[p + 1])
            elif step == 2 and p + 1 < NPAIR:
                emit_front_b(fronts[p + 1])
        emit_epilogue_row(cur, 0, cur["r0"])
        epi_pending = (cur, 1, cur["r0"] + 1)
    pending()
    emit_epilogue_row(*epi_pending)


def _build():
    if "nc" in _CACHE:
        return _CACHE["nc"], _CACHE["t"]
    nc = bass.Bass(
        "TRN2", target_bir_lowering=False, debug=False, num_devices=NCORES
    )
    t = {}
    t["xt"] = nc.dram_tensor("xt", [RPC, C, N], BF16, kind="ExternalInput")
    t["mt"] = nc.dram_tensor("mt", [RPC, C, N], BF16, kind="ExternalInput")
    t["bias_r"] = nc.dram_tensor("bias_r", [128, 2 * RPC], F32, kind="ExternalInput")
    t["nbT"] = nc.dram_tensor("nbT", [128, 2 * H * N], F32, kind="ExternalInput")
    for name in ("wqT", "wkT", "wvT", "wgT", "woT"):
        t[name] = nc.dram_tensor(name, [C, C], BF16, kind="ExternalInput")
    t["bo_row"] = nc.dram_tensor("bo_row", [1, C], BF16, kind="ExternalInput")
    t["bgn_col"] = nc.dram_tensor("bgn_col", [C, 1], F32, kind="ExternalInput")
    t["out"] = nc.dram_tensor("out", [RPC, N, C], F32, kind="ExternalOutput")

    with tile.TileContext(nc) as tc:
        with ExitStack() as ctx:
            _emit(ctx, tc, t)
    _legalize_multiwaits(nc, max_waits=1)
    _CACHE["nc"] = nc
    _CACHE["t"] = t
    return nc, t


def _prep_in_maps(q_data, m_data, bias, nonbatched_bias, wq, wk, wv, wo, bo, wg, bg):
    bf16 = mybir.dt.np(BF16)
    q_data = np.ascontiguousarray(np.asarray(q_data, np.float32))
    m_data = np.ascontiguousarray(np.asarray(m_data, np.float32))
    bias = np.asarray(bias, np.float32)
    nb = np.asarray(nonbatched_bias, np.float32)

    # pure layout prep (transposes/reshapes); all math stays on device
    consts = {
        "wqT": np.ascontiguousarray(np.asarray(wq, np.float32).T.astype(bf16)),
        "wkT": np.ascontiguousarray(np.asarray(wk, np.float32).T.astype(bf16)),
        "wvT": np.ascontiguousarray(np.asarray(wv, np.float32).T.astype(bf16)),
        "wgT": np.ascontiguousarray(np.asarray(wg, np.float32).T.astype(bf16)),
        "woT": np.ascontiguousarray(np.asarray(wo, np.float32).T.astype(bf16)),
        "bo_row": np.ascontiguousarray(np.asarray(bo, np.float32)[None, :].astype(bf16)),
        "bgn_col": np.ascontiguousarray(
            (-np.asarray(bg, np.float32))[:, None]
        ),
        # nbT[p, kc*1024 + h*256 + q] = nb[0, h, q, kc*128+p]
        "nbT": np.ascontiguousarray(
            nb[0]
            .transpose(2, 0, 1)  # [k, h, q]
            .reshape(2, 128, H, N)
            .transpose(1, 0, 2, 3)
            .reshape(128, 2 * H * N)
        ),
    }
    # bias_r[p, kc*RPC + r] = bias[0, n0+r, 0, 0, kc*128+p]
    bias_kn = bias[0, :, 0, 0, :].T.reshape(2, 128, N)  # [kc, p, n]
    in_maps = []
    for c in range(NCORES):
        n0 = c * RPC
        rows = slice(n0, n0 + RPC)
        m = dict(consts)
        m["xt"] = np.ascontiguousarray(q_data[0, rows].transpose(0, 2, 1).astype(bf16))
        m["mt"] = np.ascontiguousarray(m_data[0, rows].transpose(0, 2, 1).astype(bf16))
        m["bias_r"] = np.ascontiguousarray(
            bias_kn[:, :, rows].transpose(1, 0, 2).reshape(128, 2 * RPC)
        )
        in_maps.append(m)
    return in_maps


def kernel(**inputs) -> np.ndarray:
    global WITH_BO
    want_bo = bool(np.any(np.asarray(inputs["bo"]) != 0))
    if want_bo != WITH_BO or "nc" not in _CACHE:
        WITH_BO = want_bo
        _CACHE.clear()
    nc, _ = _build()
    in_maps = _prep_in_maps(**inputs)
    res = run_bass_kernel_spmd(nc, in_maps, core_ids=list(range(NCORES)))
    out = np.concatenate([res.results[c]["out"] for c in range(NCORES)], axis=0)
    return out.reshape(B, N, N, C).astype(np.float32)


if __name__ == "__main__":
    # smoke test against a tiny numpy reference
    rng = np.random.default_rng(0)
    inputs = {
        "q_data": rng.standard_normal((B, N, N, C), np.float32),
        "m_data": rng.standard_normal((B, N, N, C), np.float32),
        "bias": rng.standard_normal((B, N, 1, 1, N), np.float32),
        "nonbatched_bias": rng.standard_normal((1, H, N, N), np.float32),
        "wq": rng.standard_normal((C, C), np.float32) / np.sqrt(C),
        "wk": rng.standard_normal((C, C), np.float32) / np.sqrt(C),
        "wv": rng.standard_normal((C, C), np.float32) / np.sqrt(C),
        "wo": rng.standard_normal((C, C), np.float32) / np.sqrt(C),
        "bo": np.zeros((C,), np.float32),
        "wg": rng.standard_normal((C, C), np.float32) / np.sqrt(C),
        "bg": np.ones((C,), np.float32),
    }
    out = kernel(**inputs)
    print("out", out.shape, out.dtype, float(np.abs(out).max()))



# revision 38
# speedup vs baseline: 1.0422x; 1.0422x over previous
"""Trainium2 Bass kernel for gated multi-head pair attention (AlphaFold-style).

Reference computation (B=1, N=256, C=128, H=4, DH=32):
    q = (q_data @ wq.T) * DH**-0.5        # [B,N,Nq,C]
    k = m_data @ wk.T ; v = m_data @ wv.T
    logits = einsum("bnqhd,bnkhd->bnhqk", q, k) + bias + nonbatched_bias
    weight = softmax(logits, axis=-1)
    wa = einsum("bnhqk,bnkhd->bnqhd", weight, v)
    g  = sigmoid(q_data @ wg.T + bg)
    out = (wa * g).reshape(...) @ wo.T + bo

Sharding: pure data-parallel across the 8 NeuronCores along the first
residue axis (N): core c owns rows [32c, 32c+32). Params + nonbatched_bias
replicated.

On-device structure (2-row macro-iterations, software-pipelined):
  - XT/MT = q_data[n].T / m_data[n].T in SBUF [C=128 part, 256 tok] (host
    pre-transposes; pure layout prep); DMA split per row across queues and
    prefetched two pairs ahead
  - projections batched per row pair (N=512 = one full PSUM bank each):
    q2/k2 into one 2-bank tile, g2 into another; e1 = exp(-(gpre+bg))
    batched for both rows
  - logitsT[h] = k_h @ q_h.T  k-major [128 ktok, 256 q]; 4 heads via PE
    row-tiling (K=32 at row groups 32h), two per 2-bank lg tile
  - exp fused on ScalarE: exp(KEY_SCALE*logits + bias_col[k]), then one
    wide DVE multiply by precomputed exp(nonbatched_bias) per (row, kc)
  - waU[h] = v_h.T @ expT and S[h] = sum_k expT via col-tiled (M=32)
    matmuls into each row's ws tile; emul+wa/s deferred one step to avoid
    PE-FIFO head-of-line blocking of the next exp's kq matmuls
  - gate+normalize via the exp/ln ACT table set only (one table load):
    wag = waU * sigmoid(gpre+bg)/S = waU * exp(-ln(S*(1+exp(-gpre-bg))))
  - out[qc] = wag[:,qc].T @ woT (+ bo) back to q-major; contiguous DMA out
  - PSUM (8 banks): "lg" rotation 2 bufs x 2 banks (proj + logit tiles) +
    per-row "ws" tiles 2 bufs x 2 banks (v -> wa/s -> out-proj reuse);
    fronts/epilogues interleave mid-pair so ScalarE gaps stay filled

Environment workarounds (this walrus build): one sem wait max per
instruction (_legalize_multiwaits splits extras onto same-engine NOPs);
two matmuls must never write different column ranges of the same PSUM bank
(device fault) so every matmul output gets a bank-exclusive region;
custom-DVE / GPSIMD tensor ops fail codegen, hence exp(-ln(x)) reciprocal.
"""

import os
import sys

sys.path.insert(0, "/opt/trn_rl_repo")

from contextlib import ExitStack

import numpy as np

import concourse.bass as bass
import concourse.tile as tile
from concourse import mybir
from concourse.bass_utils import run_bass_kernel_spmd

B, N, C, H = 1, 256, 128, 4
DH = C // H
KEY_SCALE = DH**-0.5
NCORES = 8
RPC = int(os.getenv("KRPC", str(N // NCORES)))  # rows per core
KLEVEL = int(os.getenv("KLEVEL", "99"))  # debug feature level
WITH_BO = True  # set by kernel() per-input; bo==0 skips the bias matmuls

F32 = mybir.dt.float32
BF16 = mybir.dt.bfloat16
FP16 = mybir.dt.float16

# dtype of the post-softmax path (exp weights, v, attention matmul)
EXP_DT = BF16

_CACHE = {}


def _legalize_multiwaits(nc, max_waits=1):
    """The walrus build here encodes at most one sem wait per instruction
    ("Too many sync wait commands" otherwise). Split excess waits onto
    freshly inserted Drain instructions on the same engine just before the
    multi-wait instruction (engines execute in order, so this is
    equivalent)."""
    n_fix = 0
    for f in nc.m.functions:
        for blk in f.blocks:
            changed = False
            new_insts = []
            for inst in blk.instructions:
                si = inst.sync_info
                ow = list(si.on_wait) if (si is not None and si.on_wait) else []
                if len(ow) > max_waits:
                    head, tail = ow[:-max_waits], ow[-max_waits:]
                    while head:
                        chunk, head = head[:max_waits], head[max_waits:]
                        d = mybir.InstNoOp(
                            name=f"I-mw{nc.next_id()}", ins=[], outs=[]
                        )
                        d.engine = inst.engine
                        d.sync_info = mybir.SyncInfo(
                            on_wait=list(chunk), on_update=[]
                        )
                        new_insts.append(d)
                        n_fix += 1
                    inst.sync_info = mybir.SyncInfo(
                        on_wait=list(tail),
                        on_update=list(si.on_update) if si.on_update else [],
                    )
                    changed = True
                new_insts.append(inst)
            if changed:
                blk.instructions = new_insts
    return n_fix


def _emit(ctx: ExitStack, tc: "tile.TileContext", t):
    nc = tc.nc

    const = ctx.enter_context(tc.tile_pool(name="const", bufs=1))

    def load_const(name, shape, dtype=F32):
        sb = const.tile(shape, dtype, name=name + "_sb")
        nc.sync.dma_start(sb, t[name].ap())
        return sb

    wq_sb = load_const("wqT", [C, C], BF16)
    wk_sb = load_const("wkT", [C, C], BF16)
    wv_sb = load_const("wvT", [C, C], BF16)
    wg_sb = load_const("wgT", [C, C], BF16)
    wo_sb = load_const("woT", [C, C], BF16)
    bo_sb = load_const("bo_row", [1, C], BF16)
    bgn_sb = load_const("bgn_col", [C, 1])
    bias_sb = load_const("bias_r", [128, 2 * RPC])
    nbt_sb = load_const("nbT", [128, 2 * H * N])

    ones1_sb = const.tile([1, C], BF16)
    nc.vector.memset(ones1_sb, 1.0)
    ones32_sb = const.tile([128, DH], EXP_DT)
    nc.vector.memset(ones32_sb, 1.0)

    # one-time: exp(nonbatched_bias), k-major layout [p, kc*1024 + h*256 + q]
    enb_sb = const.tile([128, 2 * H * N], EXP_DT)
    nc.scalar.activation(enb_sb, nbt_sb, mybir.ActivationFunctionType.Exp)

    io = ctx.enter_context(tc.tile_pool(name="io", bufs=4))
    sb = ctx.enter_context(tc.tile_pool(name="sb", bufs=6))
    exps = ctx.enter_context(tc.tile_pool(name="exps", bufs=10))
    # NOTE: this walrus/HW stack faults when two matmuls write different
    # column ranges of the same PSUM bank. Every matmul output below gets a
    # bank-aligned exclusive 512-col region (2-bank [128,1024] tiles hold two
    # 256-col results at cols 0 and 512); different-partition col-tiling and
    # same-region accumulation are safe.
    # Bank layout (8 banks total): "lg" rotation 2 bufs x 2 banks = 4 banks
    # (proj qk2/gv tiles + attention logit tiles flow through it, all freed
    # by early reads), and per-row "ws" tiles 2 bufs x 2 banks = 4 banks
    # (v -> wa/s -> out-projection sequentially reuse the same banks).
    lg_ps = ctx.enter_context(tc.tile_pool(name="lg_ps", bufs=2, space="PSUM"))
    sm_ps = ctx.enter_context(tc.tile_pool(name="sm_ps", bufs=2, space="PSUM"))

    xt_ap = t["xt"].ap()
    mt_ap = t["mt"].ap()
    out_ap = t["out"]

    MM = nc.tensor.matmul
    Act = mybir.ActivationFunctionType
    NB = 512  # psum bank stride (fp32 elems)

    def pair_view(tile_ap, width):
        # [128, 1024] tile -> [128, 2, width] view of cols [0:width] and
        # [512:512+width] (the two bank-aligned result slots)
        return tile_ap.rearrange("p (b x) -> p b x", b=2)[:, :, 0:width]

    def emit_dma(fr):
        r0 = fr["r0"]
        xt_sb = io.tile([128, 2 * N], BF16, tag="xt", name="xt_sb")
        mt_sb = io.tile([128, 2 * N], BF16, tag="mt", name="mt_sb")
        # one dma_start per row so the transfer spreads over more queues
        # (each [128,256] tile is ~128 descriptors on one queue)
        for rr in range(2):
            nc.sync.dma_start(xt_sb[:, N * rr : N * rr + N], xt_ap[r0 + rr])
            nc.sync.dma_start(mt_sb[:, N * rr : N * rr + N], mt_ap[r0 + rr])
        fr["xt"], fr["mt"] = xt_sb, mt_sb

    def emit_front_a(fr):
        """q/k projections + casts: the critical chain to the next pair's
        first kq matmuls.  Emitted as early as PSUM slots allow."""
        xt_sb, mt_sb = fr["xt"], fr["mt"]
        # projections for both rows in one matmul each (N=512 = 1 full
        # bank).  qk2 2-bank tile: q2 in bank0, k2 in bank1.
        qk2_ps = lg_ps.tile([128, 2 * NB], F32, tag="lg", name="qk2_ps")
        MM(qk2_ps[:, 0:NB], lhsT=wq_sb, rhs=xt_sb, start=True, stop=True)
        MM(qk2_ps[:, NB : 2 * NB], lhsT=wk_sb, rhs=mt_sb, start=True, stop=True)
        # qk_sb layout: [q_r0 | k_r0 | q_r1 | k_r1] each N wide.  One cast
        # per row (strided across the two banks) so row0's kq matmuls can
        # start after the first cast instead of after both.
        qk_sb = sb.tile([128, 4 * N], BF16, tag="qk", name="qk_sb")
        qk2v = qk2_ps.rearrange("p (b x) -> p b x", b=2)
        for rr in range(2):
            nc.vector.tensor_copy(
                qk_sb[:, 2 * N * rr : 2 * N * rr + 2 * N].rearrange(
                    "p (b x) -> p b x", b=2
                ),
                qk2v[:, :, N * rr : N * rr + N],
            )
        fr["qk"] = qk_sb

    def emit_front_b(fr):
        """gate projection + e1, and the v projections into the ws banks."""
        xt_sb, mt_sb = fr["xt"], fr["mt"]
        gv_ps = lg_ps.tile([128, 2 * NB], F32, tag="lg", name="gv_ps")
        MM(gv_ps[:, 0:NB], lhsT=wg_sb, rhs=xt_sb, start=True, stop=True)
        # sigmoid(gpre + bg) = 1/(1 + exp(-(gpre+bg))); only exp/ln are used
        # anywhere so the ACT table set loads exactly once.  Both rows in
        # one activation pass.
        e1_sb = sb.tile([128, 2 * N], F32, tag="e1", name="e1_sb")
        nc.scalar.activation(
            e1_sb, gv_ps[:, 0 : 2 * N], Act.Exp, bias=bgn_sb, scale=-1.0
        )
        fr["e1"] = e1_sb

        # v for both rows: [v_r0 | v_r1] each [128 tok, 2*C].  The v matmuls
        # write into each row's ws tile (bank0 cols 0:C, bank1 cols 0:C);
        # the same banks are later sequentially reused for wa/s and the out
        # projection (reuse is ordered by the intervening reads).
        v_sb = sb.tile([128, 4 * C], EXP_DT, tag="v", name="v_sb")
        ws_l = []
        for rr in range(2):
            ws_ps = sm_ps.tile([128, 2 * NB], F32, tag="ws", name=f"ws{rr}_ps")
            ws_l.append(ws_ps)
            MM(ws_ps[:, 0:C], lhsT=mt_sb[:, N * rr : N * rr + 128],
               rhs=wv_sb, start=True, stop=True)
            MM(ws_ps[:, NB : NB + C],
               lhsT=mt_sb[:, N * rr + 128 : N * rr + 256],
               rhs=wv_sb, start=True, stop=True)
            nc.vector.tensor_copy(
                v_sb.rearrange("p (r c x) -> p r c x", r=2, c=2)[:, rr, :, :],
                ws_ps.rearrange("p (b x) -> p b x", b=2)[:, :, 0:C],
            )
        fr["v"] = v_sb
        fr["ws"] = ws_l

    def emit_attention_step(fr, rr, r, kc):
        """Emit kq matmuls + exp for one (row, kc); return a deferred
        emitter for the emul + wa/s matmuls.  Deferring those past the next
        step's kq matmuls avoids PE-FIFO head-of-line blocking (wa/s wait
        on the DVE emul; kq behind them would stall the next exp)."""
        qk_sb, v_sb = fr["qk"], fr["v"]
        wa_ps = fr["ws"][rr][:, 0:N]
        s_ps = fr["ws"][rr][:, NB : NB + N]
        # e2 holds exp-weights for all 4 heads of this (row, kc):
        # [p, pr*512 + hh*256 + q] = [p, 256*h + q]
        e2_sb = exps.tile([128, 4 * N], EXP_DT, tag="e", name="e2_sb")
        for pr in range(2):  # head pair (2*pr, 2*pr+1)
            lg = lg_ps.tile(
                [128, 2 * NB], F32, tag="lg", name=f"lg{rr}{kc}{pr}"
            )
            for hh in range(2):
                h = 2 * pr + hh
                # logitsT[ktok, q] = k_h @ q_h.T ; K=DH=32 at row
                # group 32h -> heads run concurrently on the PE
                MM(
                    lg[:, NB * hh : NB * hh + N],
                    lhsT=qk_sb[
                        32 * h : 32 * h + 32,
                        (2 * rr + 1) * N + 128 * kc :
                        (2 * rr + 1) * N + 128 * kc + 128,
                    ],
                    rhs=qk_sb[
                        32 * h : 32 * h + 32, 2 * rr * N : (2 * rr + 1) * N
                    ],
                    start=True,
                    stop=True,
                    tile_position=(32 * h, 0),
                )
            nc.scalar.activation(
                e2_sb[:, 2 * N * pr : 2 * N * pr + 2 * N].rearrange(
                    "p (b x) -> p b x", b=2
                ),
                pair_view(lg, N),
                Act.Exp,
                bias=bias_sb[:, kc * RPC + r : kc * RPC + r + 1],
                scale=KEY_SCALE,
            )

        def deferred():
            # one wide multiply by exp(nonbatched_bias) for all 4 heads
            nc.vector.tensor_mul(
                e2_sb, e2_sb, enb_sb[:, 1024 * kc : 1024 * kc + 1024]
            )
            for h in range(H):
                # waU (unnormalized attention @ V), col-tiled by head
                MM(
                    wa_ps[32 * h : 32 * h + 32, :],
                    lhsT=v_sb[
                        :,
                        2 * C * rr + 128 * kc + 32 * h :
                        2 * C * rr + 128 * kc + 32 * h + 32,
                    ],
                    rhs=e2_sb[:, N * h : N * h + N],
                    start=(kc == 0),
                    stop=(kc == 1),
                    tile_position=(0, 32 * h),
                    skip_group_check=True,
                )
                # S = sum_k exp, broadcast to head's partition block
                MM(
                    s_ps[32 * h : 32 * h + 32, :],
                    lhsT=ones32_sb,
                    rhs=e2_sb[:, N * h : N * h + N],
                    start=(kc == 0),
                    stop=(kc == 1),
                    tile_position=(0, 32 * h),
                    skip_group_check=True,
                )

        return deferred

    def emit_epilogue_row(fr, rr, r):
        # wag = waU * sigmoid(gpre+bg) / S = waU * exp(-ln(S * (1 + e1)))
        e1_sb = fr["e1"]
        if True:
            ws_ps = fr["ws"][rr]
            d_sb = sb.tile([128, N], F32, tag="d", name=f"d{rr}_sb")
            nc.vector.scalar_tensor_tensor(
                d_sb,
                e1_sb[:, N * rr : N * rr + N],
                1.0,
                ws_ps[:, NB : NB + N],
                mybir.AluOpType.add,
                mybir.AluOpType.mult,
            )
            nc.scalar.activation(d_sb, d_sb, Act.Ln)
            rs_sb = sb.tile([128, N], F32, tag="rs", name=f"rs{rr}_sb")
            nc.scalar.activation(rs_sb, d_sb, Act.Exp, scale=-1.0)

            wag_sb = sb.tile([128, N], BF16, tag="wag", name=f"wag{rr}")
            nc.vector.tensor_mul(wag_sb, ws_ps[:, 0:N], rs_sb)

            out_sb = sb.tile([128, 2 * C], F32, tag="out", name=f"out{rr}")
            for qc in range(2):
                # out-projection reuses the row's ws banks (wa/s fully read
                # by wag/stt at this point)
                o_ps = ws_ps[:, NB * qc : NB * qc + C]
                MM(
                    o_ps,
                    lhsT=wag_sb[:, 128 * qc : 128 * qc + 128],
                    rhs=wo_sb,
                    start=True,
                    stop=not WITH_BO,
                    skip_group_check=True,
                )
                if WITH_BO:
                    MM(
                        o_ps,
                        lhsT=ones1_sb,
                        rhs=bo_sb,
                        start=False,
                        stop=True,
                        skip_group_check=True,
                    )
            nc.vector.tensor_copy(
                out_sb.rearrange("p (b x) -> p b x", b=2),
                ws_ps.rearrange("p (b x) -> p b x", b=2)[:, :, 0:C],
            )
            # out dram [RPC, N, C]; tile is [p, qc, o] with q = qc*128 + p.
            # Last pair: split per qc chunk across two queues to halve the
            # end-of-kernel DMA drain tail (~256 descriptors per row).
            if r >= RPC - 2:
                for qc in range(2):
                    dst = bass.AP(
                        out_ap, r * N * C + qc * 128 * C, [[C, 128], [1, C]]
                    )
                    nc.sync.dma_start(dst, out_sb[:, C * qc : C * qc + C])
            else:
                dst = bass.AP(
                    out_ap, r * N * C, [[C, 128], [128 * C, 2], [1, C]]
                )
                nc.sync.dma_start(dst, out_sb)

    # Fully linearized software pipeline over steps (row, kc).  Per pair:
    #   kqA expA | wasD_prev | kqB expB | wasA | front_a' | kqC expC |
    #   wasB | front_b' | kqD expD | wasC | epi(r0) | ...next pair...
    # front_a' (q/k proj + casts) starts as soon as PSUM slots free
    # mid-pair; epilogues interleave as ACT fills; DMA is prefetched two
    # pairs ahead and split per row across queues.
    NPAIR = RPC // 2
    fronts = [{"r0": 2 * p} for p in range(NPAIR)]
    emit_dma(fronts[0])
    if NPAIR > 1:
        emit_dma(fronts[1])
    emit_front_a(fronts[0])
    emit_front_b(fronts[0])
    pending = None
    epi_pending = None
    for p in range(NPAIR):
        cur = fronts[p]
        for step in range(4):
            rr, kc = divmod(step, 2)
            nxt = emit_attention_step(cur, rr, cur["r0"] + rr, kc)
            if pending is not None:
                pending()
            pending = nxt
            if epi_pending is not None:
                emit_epilogue_row(*epi_pending)
                epi_pending = None
            if step == 1 and p + 1 < NPAIR:
                if p + 2 < NPAIR:
                    emit_dma(fronts[p + 2])
                emit_front_a(fronts[p + 1])
            elif step == 2 and p + 1 < NPAIR:
                emit_front_b(fronts[p + 1])
        emit_epilogue_row(cur, 0, cur["r0"])
        epi_pending = (cur, 1, cur["r0"] + 1)
    pending()
    emit_epilogue_row(*epi_pending)


def _build():
    if "nc" in _CACHE:
        return _CACHE["nc"], _CACHE["t"]
    nc = bass.Bass(
        "TRN2", target_bir_lowering=False, debug=False, num_devices=NCORES
    )
    t = {}
    t["xt"] = nc.dram_tensor("xt", [RPC, C, N], BF16, kind="ExternalInput")
    t["mt"] = nc.dram_tensor("mt", [RPC, C, N], BF16, kind="ExternalInput")
    t["bias_r"] = nc.dram_tensor("bias_r", [128, 2 * RPC], F32, kind="ExternalInput")
    t["nbT"] = nc.dram_tensor("nbT", [128, 2 * H * N], F32, kind="ExternalInput")
    for name in ("wqT", "wkT", "wvT", "wgT", "woT"):
        t[name] = nc.dram_tensor(name, [C, C], BF16, kind="ExternalInput")
    t["bo_row"] = nc.dram_tensor("bo_row", [1, C], BF16, kind="ExternalInput")
    t["bgn_col"] = nc.dram_tensor("bgn_col", [C, 1], F32, kind="ExternalInput")
    t["out"] = nc.dram_tensor("out", [RPC, N, C], F32, kind="ExternalOutput")

    with tile.TileContext(nc) as tc:
        with ExitStack() as ctx:
            _emit(ctx, tc, t)
    _legalize_multiwaits(nc, max_waits=1)
    _CACHE["nc"] = nc
    _CACHE["t"] = t
    return nc, t


def _prep_in_maps(q_data, m_data, bias, nonbatched_bias, wq, wk, wv, wo, bo, wg, bg):
    bf16 = mybir.dt.np(BF16)
    q_data = np.ascontiguousarray(np.asarray(q_data, np.float32))
    m_data = np.ascontiguousarray(np.asarray(m_data, np.float32))
    bias = np.asarray(bias, np.float32)
    nb = np.asarray(nonbatched_bias, np.float32)

    # pure layout prep (transposes/reshapes); all math stays on device
    consts = {
        "wqT": np.ascontiguousarray(np.asarray(wq, np.float32).T.astype(bf16)),
        "wkT": np.ascontiguousarray(np.asarray(wk, np.float32).T.astype(bf16)),
        "wvT": np.ascontiguousarray(np.asarray(wv, np.float32).T.astype(bf16)),
        "wgT": np.ascontiguousarray(np.asarray(wg, np.float32).T.astype(bf16)),
        "woT": np.ascontiguousarray(np.asarray(wo, np.float32).T.astype(bf16)),
        "bo_row": np.ascontiguousarray(np.asarray(bo, np.float32)[None, :].astype(bf16)),
        "bgn_col": np.ascontiguousarray(
            (-np.asarray(bg, np.float32))[:, None]
        ),
        # nbT[p, kc*1024 + h*256 + q] = nb[0, h, q, kc*128+p]
        "nbT": np.ascontiguousarray(
            nb[0]
            .transpose(2, 0, 1)  # [k, h, q]
            .reshape(2, 128, H, N)
            .transpose(1, 0, 2, 3)
            .reshape(128, 2 * H * N)
        ),
    }
    # bias_r[p, kc*RPC + r] = bias[0, n0+r, 0, 0, kc*128+p]
    bias_kn = bias[0, :, 0, 0, :].T.reshape(2, 128, N)  # [kc, p, n]
    in_maps = []
    for c in range(NCORES):
        n0 = c * RPC
        rows = slice(n0, n0 + RPC)
        m = dict(consts)
        m["xt"] = np.ascontiguousarray(q_data[0, rows].transpose(0, 2, 1).astype(bf16))
        m["mt"] = np.ascontiguousarray(m_data[0, rows].transpose(0, 2, 1).astype(bf16))
        m["bias_r"] = np.ascontiguousarray(
            bias_kn[:, :, rows].transpose(1, 0, 2).reshape(128, 2 * RPC)
        )
        in_maps.append(m)
    return in_maps


def kernel(**inputs) -> np.ndarray:
    global WITH_BO
    want_bo = bool(np.any(np.asarray(inputs["bo"]) != 0))
    if want_bo != WITH_BO or "nc" not in _CACHE:
        WITH_BO = want_bo
        _CACHE.clear()
    nc, _ = _build()
    in_maps = _prep_in_maps(**inputs)
    res = run_bass_kernel_spmd(nc, in_maps, core_ids=list(range(NCORES)))
    out = np.concatenate([res.results[c]["out"] for c in range(NCORES)], axis=0)
    return out.reshape(B, N, N, C).astype(np.float32)


if __name__ == "__main__":
    # smoke test against a tiny numpy reference
    rng = np.random.default_rng(0)
    inputs = {
        "q_data": rng.standard_normal((B, N, N, C), np.float32),
        "m_data": rng.standard_normal((B, N, N, C), np.float32),
        "bias": rng.standard_normal((B, N, 1, 1, N), np.float32),
        "nonbatched_bias": rng.standard_normal((1, H, N, N), np.float32),
        "wq": rng.standard_normal((C, C), np.float32) / np.sqrt(C),
        "wk": rng.standard_normal((C, C), np.float32) / np.sqrt(C),
        "wv": rng.standard_normal((C, C), np.float32) / np.sqrt(C),
        "wo": rng.standard_normal((C, C), np.float32) / np.sqrt(C),
        "bo": np.zeros((C,), np.float32),
        "wg": rng.standard_normal((C, C), np.float32) / np.sqrt(C),
        "bg": np.ones((C,), np.float32),
    }
    out = kernel(**inputs)
    print("out", out.shape, out.dtype, float(np.abs(out).max()))

